# revision 1
# baseline (speedup 1.0000x reference)
"""Trainium2 Bass kernel for CausalWanSelfAttention (block-causal + local window + sink).

Strategy (8 NeuronCores, SPMD):
  - Sequence-sharded: core c owns tokens [384c, 384c+384).
  - Each core projects its tokens to Q/K/V (bf16 matmuls), RMS-norm + RoPE
    folded into host-precomputed tables, K/V AllGathered across cores.
  - Attention: the mask is all-or-nothing at 256-token frame granularity, so
    each query block attends a dense context = sink frame (256 tokens, static
    slot) + a 4-slot window of the gathered K/V whose base is derived from
    partition_id at runtime. Invalid/duplicate context rows are suppressed
    with a per-core additive bias (-1e4) fused into the exp activation.
  - Softmax denominator via a ones-column matmul accumulated in PSUM
    (deferred after PV so the PE stream never stalls on the normalize chain).
  - Repeat variants are software-pipelined one rep deep: section k emits
    proj(k)+AllGather(k) then attention(k-1)+out-proj(k-1), with parity
    double-buffered collective DRAM tensors, so collectives and the PE
    stream fully overlap and the PE stays HAM-warm.
"""
import os
import time
import numpy as np

import concourse.bass as bass
import concourse.tile as tile
from concourse import bacc, mybir
from concourse.bass_interp import get_hw_module

F32 = mybir.dt.float32
F32R = mybir.dt.float32r
BF16 = mybir.dt.bfloat16
AF = mybir.ActivationFunctionType
ALU = mybir.AluOpType

DIM = 1536
NH = 12
HD = 128
S = 3072
NC = 8
T = S // NC          # 384 tokens per core
NT = DIM // 128      # 12 o/i tiles
C = HD // 2          # 64 rope pairs
CTX = 1792           # 256 sink + 4*384 window
NCT = CTX // 128     # 14 ctx tiles
NFPB = 3
LOCAL = 6
SINK = 1
EPS = 1e-6
MASK_NEG = -1.0e4

# inputs that are identical on every core (replicated on the device mesh)
REPLICATED = {"wq", "wk", "wv", "wo", "bqr", "bkr", "onesc", "onesrow"}

_CACHE = {}
LAST_RESULT = None


def _emit(tc, repeat=1):
    nc = tc.nc
    from contextlib import ExitStack

    xt_d = nc.dram_tensor("xt", [DIM, T], BF16, kind="ExternalInput").ap()
    wq_d = nc.dram_tensor("wq", [DIM, DIM], BF16, kind="ExternalInput").ap()
    wk_d = nc.dram_tensor("wk", [DIM, DIM], BF16, kind="ExternalInput").ap()
    wv_d = nc.dram_tensor("wv", [DIM, DIM], BF16, kind="ExternalInput").ap()
    wo_d = nc.dram_tensor("wo", [DIM, DIM], BF16, kind="ExternalInput").ap()
    bq_d = nc.dram_tensor("bqr", [128, NT], F32, kind="ExternalInput").ap()
    bk_d = nc.dram_tensor("bkr", [128, NT], F32, kind="ExternalInput").ap()
    qcos_d = nc.dram_tensor("qcos", [DIM, T], F32, kind="ExternalInput").ap()
    qsin_d = nc.dram_tensor("qsin", [DIM, T], F32, kind="ExternalInput").ap()
    kcos_d = nc.dram_tensor("kcos", [DIM, T], F32, kind="ExternalInput").ap()
    ksin_d = nc.dram_tensor("ksin", [DIM, T], F32, kind="ExternalInput").ap()
    ones_d = nc.dram_tensor("onesc", [128, 1], BF16, kind="ExternalInput").ap()
    onesrow_d = nc.dram_tensor("onesrow", [1, 128], F32, kind="ExternalInput").ap()
    mask_d = nc.dram_tensor("maskb", [128, NCT], F32, kind="ExternalInput").ap()
    y_d = nc.dram_tensor("y", [T, DIM], F32, kind="ExternalOutput").ap()

    # parity double-buffered collective buffers (rep k uses k % 2)
    agk_in = [nc.dram_tensor(f"agk_in{p}", [DIM, T], BF16).ap() for p in range(2)]
    agv_in = [nc.dram_tensor(f"agv_in{p}", [T, DIM], BF16).ap() for p in range(2)]
    agk_out = [nc.dram_tensor(f"agk_out{p}", [NC * DIM, T], BF16,
                              addr_space="Shared").ap() for p in range(2)]
    agv_out = [nc.dram_tensor(f"agv_out{p}", [NC * T, DIM], BF16,
                              addr_space="Shared").ap() for p in range(2)]

    with ExitStack() as top:
        persist = top.enter_context(tc.tile_pool(name="persist", bufs=1))
        wpool = top.enter_context(tc.tile_pool(name="wband", bufs=16))
        xpool = top.enter_context(tc.tile_pool(name="xp", bufs=2))
        prepool = top.enter_context(tc.tile_pool(name="prep", bufs=1))
        tabpool = top.enter_context(tc.tile_pool(name="tabp", bufs=3))
        tmp = top.enter_context(tc.tile_pool(name="tmp", bufs=3))
        small = top.enter_context(tc.tile_pool(name="small", bufs=2))
        qtrp = top.enter_context(tc.tile_pool(name="qtrp", bufs=24))
        atnp = top.enter_context(tc.tile_pool(name="atnp", bufs=24))
        arawp = top.enter_context(tc.tile_pool(name="arawp", bufs=3))
        rdbp = top.enter_context(tc.tile_pool(name="rdbp", bufs=2))
        prp = top.enter_context(tc.tile_pool(name="probs", bufs=16))
        kvp = top.enter_context(tc.tile_pool(name="kvp", bufs=3))
        outp = top.enter_context(tc.tile_pool(name="outp", bufs=2))

        ps_acc = top.enter_context(tc.tile_pool(name="psacc", bufs=2, space="PSUM"))
        ps_sw = top.enter_context(tc.tile_pool(name="pssw", bufs=1, space="PSUM"))
        ps_sm = top.enter_context(tc.tile_pool(name="pssm", bufs=2, space="PSUM"))
        ps_s = top.enter_context(tc.tile_pool(name="pss", bufs=2, space="PSUM"))
        ps_o = top.enter_context(tc.tile_pool(name="pso", bufs=1, space="PSUM"))

        ones_sb = persist.tile([128, 1], BF16, tag="ones", name="ones")
        nc.sync.dma_start(ones_sb[:], ones_d)
        onesrow_sb = persist.tile([1, 128], F32R, tag="onesrow", name="onesrow")
        nc.sync.dma_start(onesrow_sb[:], onesrow_d.bitcast(F32R))
        mask_sb = persist.tile([128, NCT], F32, tag="mask", name="mask")
        nc.sync.dma_start(mask_sb[:], mask_d)
        bq_sb = persist.tile([128, NT], F32, tag="bq", name="bq")
        nc.sync.dma_start(bq_sb[:], bq_d)
        bk_sb = persist.tile([128, NT], F32, tag="bk", name="bk")
        nc.sync.dma_start(bk_sb[:], bk_d)
        eps_sb = persist.tile([1, 1], F32, tag="eps", name="eps")
        nc.vector.memset(eps_sb[:], EPS)

        # window base (slots) from partition id: 2*(pid>=4) + 2*(pid>=6)
        e = nc.sync
        pid = e.partition_id()
        r1 = e.alloc_register("wge4")
        e.reg_alu(r1, pid, 3, ALU.subtract)
        e.reg_alu(r1, r1, 0, ALU.max)
        e.reg_alu(r1, r1, 1, ALU.min)
        r2 = e.alloc_register("wge6")
        e.reg_alu(r2, pid, 5, ALU.subtract)
        e.reg_alu(r2, r2, 0, ALU.max)
        e.reg_alu(r2, r2, 1, ALU.min)
        e.reg_alu(r1, r1, r2, ALU.add)
        e.reg_alu(r1, r1, 2, ALU.mult)
        w_sv = e.snap(r1, donate=True, min_val=0, max_val=4)

        ctx = dict(
            nc=nc, w_sv=w_sv, ones_sb=ones_sb,
            onesrow_sb=onesrow_sb, mask_sb=mask_sb, bq_sb=bq_sb, bk_sb=bk_sb,
            eps_sb=eps_sb, wpool=wpool, xpool=xpool, prepool=prepool,
            tabpool=tabpool, tmp=tmp, small=small, qtrp=qtrp, atnp=atnp, arawp=arawp,
            rdbp=rdbp, prp=prp, kvp=kvp, outp=outp, ps_acc=ps_acc,
            ps_sw=ps_sw, ps_sm=ps_sm, ps_s=ps_s, ps_o=ps_o,
            xt_d=xt_d, wq_d=wq_d, wk_d=wk_d, wv_d=wv_d, wo_d=wo_d,
            qcos_d=qcos_d, qsin_d=qsin_d, kcos_d=kcos_d, ksin_d=ksin_d,
            y_d=y_d, agk_in=agk_in, agv_in=agv_in,
            agk_out=agk_out, agv_out=agv_out,
        )

        prev = None
        for k in range(repeat):
            prev = _emit_section(tc, k, prev, ctx)
        _emit_attn(tc, prev, ctx)


def _proj_qk(tc, k, ctx, tagp, w_dram, b_sb, cos_dram, sin_dram, dst_tiles, ag_dst):
    """Project + RMS-norm + RoPE one of Q/K.  dst_tiles: 12 bf16 [128,T] tiles."""
    nc = ctx["nc"]
    R = f"_{tagp}{k}"
    wpool, prepool, tabpool, tmp = ctx["wpool"], ctx["prepool"], ctx["tabpool"], ctx["tmp"]
    ps_acc, ps_sw, ps_sm = ctx["ps_acc"], ctx["ps_sw"], ctx["ps_sm"]
    xt_sb = ctx["xt_sb"]
    ones_sb, onesrow_sb, eps_sb = ctx["ones_sb"], ctx["onesrow_sb"], ctx["eps_sb"]

    bands = [wpool.tile([128, DIM], BF16, tag="w", name=f"wb{_i}" + R)
             for _i in range(NT)]
    for i in range(NT):
        nc.sync.dma_start(bands[i][:], w_dram[128 * i:128 * (i + 1), :])
    ssum_ps = ps_sm.tile([1, T], F32, tag="sm", name="ss" + R)
    pres = []
    for n in range(NT):
        ps = ps_acc.tile([128, T], F32, tag="acc", name=f"ps{n}" + R)
        for i in range(NT):
            nc.tensor.matmul(ps[:], bands[i][:, 128 * n:128 * (n + 1)],
                             xt_sb[:, i, :], start=(i == 0), stop=(i == NT - 1))
        pre = prepool.tile([128, T], BF16, tag=f"pre{n}", name=f"pre{n}" + R)
        nc.scalar.activation(pre[:], ps[:], AF.Identity, bias=b_sb[:, n:n + 1])
        sq = tmp.tile([128, T], BF16, tag="sq", name=f"sq{n}" + R)
        nc.scalar.activation(sq[:], ps[:], AF.Square, bias=b_sb[:, n:n + 1])
        nc.tensor.matmul(ssum_ps[:], ones_sb[:], sq[:],
                         start=(n == 0), stop=(n == NT - 1))
        pres.append(pre)
    small = ctx["small"]
    srt = small.tile([1, T], F32, tag="srt", name="srt" + R)
    nc.scalar.activation(srt[:], ssum_ps[:], AF.Sqrt, bias=eps_sb[:], scale=1.0 / DIM)
    rd = small.tile([1, T], F32, tag="rd", name="rd" + R)
    nc.vector.reciprocal(rd[:], srt[:])
    rd_r = small.tile([1, T], F32R, tag="rdr", name="rdr" + R)
    nc.vector.tensor_copy(rd_r[:], rd[:])
    rd_ps = ps_sw.tile([128, T], F32, tag="sw", name="rdps" + R)
    nc.tensor.matmul(rd_ps[:], onesrow_sb[:], rd_r[:], start=True, stop=True)
    rd_b = ctx["rdbp"].tile([128, T], F32, tag="rdb", name="rdb" + R)
    nc.vector.tensor_copy(rd_b[:], rd_ps[:])
    for n in range(NT):
        # de-interleaved rope: head rows [0:64]=real pairs, [64:128]=imag pairs
        # (weights/bias/tables permuted host-side).  rot = pre*cos + blockswap(pre*sin)
        cos_t = tabpool.tile([128, T], F32, tag="cost", name=f"cos{n}" + R)
        nc.scalar.dma_start(cos_t[:], cos_dram[128 * n:128 * (n + 1), :])
        sin_t = tabpool.tile([128, T], F32, tag="sint", name=f"sin{n}" + R)
        nc.scalar.dma_start(sin_t[:], sin_dram[128 * n:128 * (n + 1), :])
        m1 = tmp.tile([128, T], F32, tag="m1", name=f"m1_{n}" + R)
        nc.vector.tensor_mul(m1[:], pres[n][:], cos_t[:])
        m2 = tmp.tile([128, T], F32, tag="m2", name=f"m2_{n}" + R)
        nc.vector.tensor_mul(m2[:], pres[n][:], sin_t[:])
        msw = tmp.tile([128, T], F32, tag="msw", name=f"msw{n}" + R)
        nc.sync.dma_start(msw[0:64, :], m2[64:128, :])
        nc.sync.dma_start(msw[64:128, :], m2[0:64, :])
        nc.vector.tensor_add(m1[:], m1[:], msw[:])
        nc.vector.tensor_mul(dst_tiles[n][:], m1[:], rd_b[:])
        if ag_dst is not None:
            nc.sync.dma_start(
                ag_dst.rearrange("(n d) t -> d n t", n=NT)[:, n, :],
                dst_tiles[n][:])


def _emit_section(tc, k, prev, ctx):
    """Emit proj(k) + AllGathers(k), then attention(k-1) + out-proj(k-1)."""
    nc = ctx["nc"]
    p = k % 2
    R = f"_r{k}"

    xt_sb = ctx["xpool"].tile([128, NT, T], BF16, tag="xt", name="xt" + R)
    nc.sync.dma_start(xt_sb[:], ctx["xt_d"].rearrange("(n d) t -> d n t", n=NT))
    ctx["xt_sb"] = xt_sb

    # K projection -> agk_in[p] -> AllGather
    kdst = [ctx["kvp"].tile([128, T], BF16, tag="kd", name=f"kd{_i}" + R)
            for _i in range(NT)]
    _proj_qk(tc, k, ctx, "k", ctx["wk_d"], ctx["bk_sb"], ctx["kcos_d"],
             ctx["ksin_d"], kdst, ctx["agk_in"][p])
    nc.gpsimd.collective_compute(
        "AllGather", mybir.AluOpType.bypass,
        ins=[ctx["agk_in"][p]], outs=[ctx["agk_out"][p]],
        replica_groups=[list(range(NC))])

    # V projection (natural layout) -> agv_in[p] -> AllGather
    vbands = [ctx["wpool"].tile([128, DIM], BF16, tag="w", name=f"vb{_i}" + R)
              for _i in range(NT)]
    for i in range(NT):
        nc.sync.dma_start(vbands[i][:], ctx["wv_d"][128 * i:128 * (i + 1), :])
    for tc_i in range(3):
        for oc in range(3):
            ps = ctx["ps_acc"].tile([128, 512], F32, tag="acc", name=f"vps{tc_i}_{oc}" + R)
            for i in range(NT):
                nc.tensor.matmul(ps[:], xt_sb[:, i, 128 * tc_i:128 * (tc_i + 1)],
                                 vbands[i][:, 512 * oc:512 * (oc + 1)],
                                 start=(i == 0), stop=(i == NT - 1))
            vsb = ctx["tmp"].tile([128, 512], BF16, tag="vsb", name=f"vsb{tc_i}_{oc}" + R)
            nc.vector.tensor_copy(vsb[:], ps[:])
            nc.sync.dma_start(
                ctx["agv_in"][p][128 * tc_i:128 * (tc_i + 1), 512 * oc:512 * (oc + 1)],
                vsb[:])
    nc.gpsimd.collective_compute(
        "AllGather", mybir.AluOpType.bypass,
        ins=[ctx["agv_in"][p]], outs=[ctx["agv_out"][p]],
        replica_groups=[list(range(NC))])

    # Q projection (stays in SBUF)
    qt_rot = [ctx["qtrp"].tile([128, T], BF16, tag="qtr", name=f"qtr{n}" + R)
              for n in range(NT)]
    _proj_qk(tc, k, ctx, "q", ctx["wq_d"], ctx["bq_sb"], ctx["qcos_d"],
             ctx["qsin_d"], qt_rot, None)

    # attention + out-proj for the previous rep (pipelined one stage back)
    if prev is not None:
        _emit_attn(tc, prev, ctx)

    return dict(k=k, qt_rot=qt_rot)


def _emit_attn(tc, state, ctx):
    """Attention + output projection for rep `state['k']`."""
    nc = ctx["nc"]
    k = state["k"]
    p = k % 2
    qt_rot = state["qt_rot"]
    R = f"_a{k}"
    w_sv = ctx["w_sv"]
    ps_s, ps_o, ps_sm, ps_sw = ctx["ps_s"], ctx["ps_o"], ctx["ps_sm"], ctx["ps_sw"]
    prp, kvp, tmp = ctx["prp"], ctx["kvp"], ctx["tmp"]
    mask_sb, ones_sb, onesrow_sb = ctx["mask_sb"], ctx["ones_sb"], ctx["onesrow_sb"]

    # prefetch Wo bands
    obands = [ctx["wpool"].tile([128, DIM], BF16, tag="w", name=f"owb{_i}" + R)
              for _i in range(NT)]
    for i in range(NT):
        nc.sync.dma_start(obands[i][:], ctx["wo_d"][128 * i:128 * (i + 1), :])

    agk4 = ctx["agk_out"][p].rearrange("(r n d) t -> d r n t", r=NC, n=NT)
    agv4 = ctx["agv_out"][p].rearrange("(r b q) o -> q r b o", r=NC, b=3)

    attnT = [ctx["atnp"].tile([128, T], BF16, tag="atn", name=f"atn{n}" + R)
             for n in range(NH)]

    # deferred normalize chains: the broadcast matmul of head h is emitted one
    # head later so the PE never waits on the recip chain
    pending = []

    def emit_bcast(item):
        hh, rdr_t, araw_t = item
        rd_ps = ps_sw.tile([128, T], F32, tag="sw", name=f"ardps{hh}" + R)
        nc.tensor.matmul(rd_ps[:], onesrow_sb[:], rdr_t[:], start=True, stop=True)
        rd_bs = ctx["rdbp"].tile([128, T], F32, tag="rdb", name=f"ardb{hh}" + R)
        nc.vector.tensor_copy(rd_bs[:], rd_ps[:])
        nc.vector.tensor_mul(attnT[hh][:], araw_t[:], rd_bs[:])

    for h in range(NH):
        kt = kvp.tile([128, CTX], BF16, tag="kt", name=f"kt{h}" + R)
        nc.sync.dma_start(kt[:, 0:256], agk4[:, 0, h, 0:256])
        nc.sync.dma_start(kt[:, 256:CTX].rearrange("q (r t) -> q r t", r=4),
                          agk4[:, bass.ds(w_sv, 4), h, :])
        vt = kvp.tile([128, NCT, 128], BF16, tag="vt", name=f"vt{h}" + R)
        nc.sync.dma_start(vt[:, 0:2, :], agv4[:, 0, 0:2, 128 * h:128 * (h + 1)])
        nc.sync.dma_start(vt[:, 2:NCT, :].rearrange("q (r b) o -> q r b o", r=4),
                          agv4[:, bass.ds(w_sv, 4), :, 128 * h:128 * (h + 1)])

        ps_ot = ps_o.tile([128, T], F32, tag="o", name=f"pso{h}" + R)
        prs = []
        for ct in range(NCT):
            ps_st = ps_s.tile([128, T], F32, tag="s", name=f"s{h}_{ct}" + R)
            nc.tensor.matmul(ps_st[:], kt[:, 128 * ct:128 * (ct + 1)], qt_rot[h][:],
                             start=True, stop=True)
            pr = prp.tile([128, T], BF16, tag="pr", name=f"pr{h}_{ct}" + R)
            nc.scalar.activation(pr[:], ps_st[:], AF.Exp,
                                 bias=mask_sb[:, ct:ct + 1], scale=1.0)
            nc.tensor.matmul(ps_ot[:], vt[:, ct, :], pr[:],
                             start=(ct == 0), stop=(ct == NCT - 1))
            prs.append(pr)
        # copy PV accumulator out immediately to free the single PSUM bank
        araw = ctx["arawp"].tile([128, T], F32, tag="araw", name=f"araw{h}" + R)
        nc.vector.tensor_copy(araw[:], ps_ot[:])
        # deferred denominator matmuls (back-to-back, no PE stall)
        ps_d = ps_sm.tile([1, T], F32, tag="sm", name=f"psd{h}" + R)
        for ct in range(NCT):
            nc.tensor.matmul(ps_d[:], ones_sb[:], prs[ct][:],
                             start=(ct == 0), stop=(ct == NCT - 1))
        rd = ctx["small"].tile([1, T], F32, tag="rd", name=f"rda{h}" + R)
        nc.vector.reciprocal(rd[:], ps_d[:])
        rd_r = ctx["small"].tile([1, T], F32R, tag="rdr", name=f"rdra{h}" + R)
        nc.vector.tensor_copy(rd_r[:], rd[:])
        pending.append((h, rd_r, araw))
        if len(pending) >= 2:
            emit_bcast(pending.pop(0))
    while pending:
        emit_bcast(pending.pop(0))

    # output projection: y[t, o] = sum_i attnT[i][t] * woT[i, o]
    for tc_i in range(3):
        for oc in range(3):
            ps = ctx["ps_acc"].tile([128, 512], F32, tag="acc", name=f"yps{tc_i}_{oc}" + R)
            for i in range(NT):
                nc.tensor.matmul(ps[:], attnT[i][:, 128 * tc_i:128 * (tc_i + 1)],
                                 obands[i][:, 512 * oc:512 * (oc + 1)],
                                 start=(i == 0), stop=(i == NT - 1))
            osb = ctx["outp"].tile([128, 512], F32, tag="ob", name=f"osb{tc_i}_{oc}" + R)
            nc.vector.tensor_copy(osb[:], ps[:])
            nc.sync.dma_start(
                ctx["y_d"][128 * tc_i:128 * (tc_i + 1), 512 * oc:512 * (oc + 1)],
                osb[:])


def _build(repeat=1):
    key = ("nc", repeat)
    if key in _CACHE:
        return _CACHE[key]
    nc = bacc.Bacc("TRN2", target_bir_lowering=False, debug=False,
                   enable_asserts=False, num_devices=NC)
    with tile.TileContext(nc) as tc:
        _emit(tc, repeat)
    nc.compile()
    nc.m = get_hw_module(nc.m)
    _CACHE[key] = nc
    return nc


# ---------------------------------------------------------------------------
# host-side input preparation
# ---------------------------------------------------------------------------

def _pos_table(tab, f, h, w):
    cf = C - 2 * (C // 3)
    ch = C // 3
    tf = np.broadcast_to(tab[:f, :cf][:, None, None, :], (f, h, w, cf))
    th = np.broadcast_to(tab[:h, cf:cf + ch][None, :, None, :], (f, h, w, ch))
    tw = np.broadcast_to(tab[:w, cf + ch:][None, None, :, :], (f, h, w, ch))
    return np.concatenate([tf, th, tw], axis=-1).reshape(f * h * w, C)


def _rope_tables(cosP, sinP, g, scale):
    """(cosT, sinT) [S, DIM] in the de-interleaved head layout, folding g and
    the score scale.

    Per head, row j<64 holds original dim 2j (real), row 64+j holds dim 2j+1
    (imag).  Device computes rot = pre*cosT + blockswap(pre*sinT) where
    blockswap exchanges the two 64-row halves of each head tile."""
    cosT = np.empty((S, DIM), np.float32)
    sinT = np.empty((S, DIM), np.float32)
    for n in range(NH):
        gh = g[128 * n:128 * (n + 1)]
        lo = slice(128 * n, 128 * n + 64)
        hi = slice(128 * n + 64, 128 * n + 128)
        cosT[:, lo] = cosP * gh[0::2][None, :] * scale
        cosT[:, hi] = cosP * gh[1::2][None, :] * scale
        # b = pre*sinT; out_real = a_lo + b_hi_swapped  -> sinT_hi = -sin*g_odd
        #               out_imag = a_hi + b_lo_swapped  -> sinT_lo = +sin*g_even
        sinT[:, lo] = sinP * gh[0::2][None, :] * scale
        sinT[:, hi] = -sinP * gh[1::2][None, :] * scale
    return cosT, sinT


def _perm_cols_per_head(w):
    """Permute the per-head output dims of [in, DIM] (or [DIM]) arrays into the
    de-interleaved layout: new col j<64 <- 2j, new col 64+j <- 2j+1."""
    perm = np.empty(HD, np.int64)
    perm[0:64] = np.arange(64) * 2
    perm[64:128] = np.arange(64) * 2 + 1
    full = np.concatenate([128 * n + perm for n in range(NH)])
    return w[..., full]


def _mask_for_core(c):
    qb = c // 2
    frame = np.arange(S) // 256
    blk = frame // NFPB

    def allowed(kk):
        return (blk[kk] <= qb) & (((qb - blk[kk]) * NFPB < LOCAL) | (frame[kk] < SINK))

    m = np.full(CTX, MASK_NEG, np.float32)
    if qb >= 2:
        m[0:256] = 0.0
    wbase = 2 * max(qb - 1, 0)
    tok = np.arange(T * wbase, T * wbase + 1536)
    m[256:] = np.where(allowed(tok), 0.0, MASK_NEG)
    return np.ascontiguousarray(m.reshape(NCT, 128).T)  # [128, NCT]


def _prep_in_maps(x, Wq, bq, Wk, bk, Wv, bv, Wo, bo, gq, gk, freqs_cos, freqs_sin, f, h, w):
    x = np.asarray(x, np.float32)
    f, h, w = int(f), int(h), int(w)
    cosP = _pos_table(np.asarray(freqs_cos, np.float32), f, h, w)
    sinP = _pos_table(np.asarray(freqs_sin, np.float32), f, h, w)

    qcosT, qsinT = _rope_tables(cosP, sinP, np.asarray(gq, np.float32), HD ** -0.5)
    kcosT, ksinT = _rope_tables(cosP, sinP, np.asarray(gk, np.float32), 1.0)

    import ml_dtypes
    BF = ml_dtypes.bfloat16
    wq_t = np.ascontiguousarray(
        _perm_cols_per_head(np.asarray(Wq, np.float32).T)).astype(BF)
    wk_t = np.ascontiguousarray(
        _perm_cols_per_head(np.asarray(Wk, np.float32).T)).astype(BF)
    wv_t = np.ascontiguousarray(np.asarray(Wv, np.float32).T).astype(BF)
    wo_t = np.ascontiguousarray(np.asarray(Wo, np.float32).T).astype(BF)
    bq_r = np.ascontiguousarray(
        _perm_cols_per_head(np.asarray(bq, np.float32)).reshape(NT, 128).T)
    bk_r = np.ascontiguousarray(
        _perm_cols_per_head(np.asarray(bk, np.float32)).reshape(NT, 128).T)

    onesc = np.ones((128, 1), BF)
    onesrow = np.ones((1, 128), np.float32)

    xs = x[0]  # [S, DIM]
    in_maps = []
    for c in range(NC):
        xt_c = np.ascontiguousarray(xs[T * c:T * (c + 1), :].T).astype(BF)
        sl = slice(T * c, T * (c + 1))
        in_maps.append(dict(
            xt=xt_c, wq=wq_t, wk=wk_t, wv=wv_t, wo=wo_t,
            bqr=bq_r, bkr=bk_r,
            qcos=np.ascontiguousarray(qcosT[sl].T), qsin=np.ascontiguousarray(qsinT[sl].T),
            kcos=np.ascontiguousarray(kcosT[sl].T), ksin=np.ascontiguousarray(ksinT[sl].T),
            onesc=onesc, onesrow=onesrow, maskb=_mask_for_core(c),
        ))

    bo_eff = np.asarray(bo, np.float32) + np.asarray(bv, np.float32) @ np.asarray(Wo, np.float32).T
    return in_maps, bo_eff


def _assemble(per_core_y, bo_eff):
    out = np.concatenate(per_core_y, axis=0)  # [S, DIM]
    out = out + bo_eff[None, :]
    return out[None].astype(np.float32)


# ---------------------------------------------------------------------------
# execution (PJRT shard_map; replicated specs for weights)
# ---------------------------------------------------------------------------

def _make_runner(nc):
    import jax
    from jax.sharding import Mesh, PartitionSpec
    try:
        from jax.experimental.shard_map import shard_map
    except ImportError:
        from jax.shard_map import shard_map
    from concourse.bass2jax import _bass_exec_p, install_neuronx_cc_hook, partition_id_tensor

    install_neuronx_cc_hook()
    partition_name = nc.partition_id_tensor.name if nc.partition_id_tensor else None
    in_names, out_names, out_avals = [], [], []
    for alloc in nc.m.functions[0].allocations:
        if not isinstance(alloc, mybir.MemoryLocationSet):
            continue
        name = alloc.memorylocations[0].name
        if alloc.kind == "ExternalInput":
            if name != partition_name:
                in_names.append(name)
        elif alloc.kind == "ExternalOutput":
            out_names.append(name)
            out_avals.append(jax.core.ShapedArray(tuple(alloc.tensor_shape),
                                                  mybir.dt.np(alloc.dtype)))
    n_params = len(in_names)
    all_in_names = list(in_names) + out_names
    if partition_name is not None:
        all_in_names.append(partition_name)

    def _body(*args):
        ins = list(args[:n_params])
        zouts = list(args[n_params:])
        extra = [partition_id_tensor()] if partition_name is not None else []
        outs = _bass_exec_p.bind(
            *ins, *zouts, *extra,
            out_avals=tuple(out_avals),
            in_names=tuple(all_in_names),
            out_names=tuple(out_names),
            lowering_input_output_aliases=(),
            sim_require_finite=False,
            sim_require_nnan=False,
            nc=nc,
        )
        return tuple(outs)

    import numpy as _np
    devices = jax.devices()[:NC]
    mesh = Mesh(_np.asarray(devices), ("core",))
    in_specs = tuple(
        PartitionSpec() if name in REPLICATED else PartitionSpec("core")
        for name in in_names
    ) + (PartitionSpec("core"),) * len(out_names)
    out_specs = (PartitionSpec("core"),) * len(out_names)
    fn = jax.jit(shard_map(_body, mesh=mesh, in_specs=in_specs,
                           out_specs=out_specs, check_rep=False))
    return fn, in_names, out_names, out_avals


def _prepare_args(in_maps, in_names, out_avals):
    import jax
    args = []
    for i, name in enumerate(in_names):
        if name in REPLICATED:
            args.append(in_maps[0][name])
        else:
            args.append(np.concatenate([np.asarray(m[name]) for m in in_maps], axis=0))
    for a in out_avals:
        args.append(np.zeros((NC * a.shape[0], *a.shape[1:]), a.dtype))
    return [jax.device_put(a) for a in args]


def _run(nc, in_maps):
    import jax
    key = ("runner", id(nc))
    if key not in _CACHE:
        _CACHE[key] = _make_runner(nc)
    fn, in_names, out_names, out_avals = _CACHE[key]
    args = _prepare_args(in_maps, in_names, out_avals)
    outs = fn(*args)
    jax.block_until_ready(outs)
    results = []
    for c in range(NC):
        r = {}
        for i, name in enumerate(out_names):
            r[name] = np.asarray(outs[i]).reshape(NC, *out_avals[i].shape)[c]
        results.append(r)
    return results


def kernel(**inputs):
    global LAST_RESULT
    in_maps, bo_eff = _prep_in_maps(**inputs)
    nc = _build()
    results = _run(nc, in_maps)
    LAST_RESULT = results
    return _assemble([results[c]["y"] for c in range(NC)], bo_eff)

